# revision 22
# baseline (speedup 1.0000x reference)
"""Trainium2 Bass kernel for the C-LIF spiking-neuron forward pass.

Problem: x [16, 8192, 200] fp32, scalar decays dm=0.9, ds=0.6, VTH=0.5.
Per neuron, over time t:
    M = dm*(M + x_t); S = ds*(S + x_t); E = dm*E + o_prev*VTH
    u = M - S - E;    o_t = (u - VTH > 0)

Reformulation (exact in real arithmetic; fp32 roundings differ from the
reference by ~1 ulp, flipping only a handful of spikes):
    2*(M-S) = 0.6*y2 where y2 = two-pole IIR cascade (dm, ds) on x.
    With v := y1 - 0.4, h := 0.6*y2 - 1, F := E/VTH, and the prescale
    xa := 0.6*x - 0.04 (constants absorbed into shifted states):
        v[t] = dm*v[t-1] + xa[t]        v[-1] = -0.4
        h[t] = ds*h[t-1] + v[t]         h[-1] = -1
        F[t] = dm*F[t-1] + o[t-1]       F[-1] = o[-1] = 0
        o[t] = (F[t] < h[t])

The whole recurrence runs as ONE hand-built custom DVE uop program at
1 element/cycle: neuron rows are processed two-at-a-time, interleaved
element-wise, so the DVE's NEXT_ALU_OUT_A/B backward feedback paths
(2-cycle latency) deliver exactly the t-1 state with zero bubbles.
Per-pair state reset rides the SUB_DIM_DONE trigger; the first two
elements of each segment run a boundary uop that substitutes the
initial state for the stale feedback flops.

Per core: DMA-in -> ScalarE/GPSIMD affine+pair-interleave -> fused DVE
scan (bf16 spikes out) -> DMA-out.  The kernel is DMA-bound (~20 MB of
HBM traffic per core).  Host side: pair-uninterleave + upcast to fp32.

Sharding: 131072 neuron rows split evenly across 8 cores (data
parallel, no cross-device communication).
"""

import numpy as np

# ---------------------------------------------------------------- constants
B, N, T = 16, 8192, 200
N_CORES = 8
ROWS = B * N                      # 131072 neuron rows
ROWS_PER_CORE = ROWS // N_CORES   # 16384
G = ROWS_PER_CORE // 128          # 128 groups of 128 neurons
NPAIR = G // 2                    # 64 interleaved pairs
SEG = 2 * T                       # 400: elements per pair segment
GB = 8                            # groups per DMA batch
NB = G // GB                      # 16 DMA-in batches
PB = 16                           # pairs per DVE op
ND = NPAIR // PB                  # 4 DVE ops / out-DMA chunks

DM = np.float32(0.9)
DS = np.float32(0.6)
GAIN = np.float32(0.6)            # 2*(dm-ds)
ABIAS = np.float32(-0.04)         # 0.4*(dm-1): affine bias for xa
V_INIT = np.float32(-0.4)
C2_VAL = np.float32(DM * V_INIT)  # dm*v_init: boundary vm value

_cached = {}

# ------------------------------------------------------------ custom DVE op
LANE_XA, LANE_DM, LANE_DS, LANE_ZERO, LANE_V, LANE_H = 0, 1, 2, 3, 4, 5


def _build_lif_uops():
    from concourse.dve_uop import (
        ENABLE,
        AluInp,
        AluOp,
        DelayInp,
        InpSel,
        OutPath,
        OutSel,
        Trigger,
        UopConfig,
        UopDpConfig,
    )

    def datapath(boundary):
        b = [UopDpConfig() for _ in range(8)]
        for st in range(8):
            b[st].pass_through_delay(LANE_XA, LANE_DM, LANE_DS, LANE_ZERO)
        b[2].enable_delay_from_src(DelayInp.PREV_ALU_OUT, LANE_V)
        for st in range(3, 8):
            b[st].pass_through_delay(LANE_V)
        b[4].enable_delay_from_src(DelayInp.PREV_ALU_OUT, LANE_H)
        for st in range(5, 8):
            b[st].pass_through_delay(LANE_H)

        if boundary:
            b[0].enable_alu(AluOp.BYPASS, AluInp.PREV_ALU_OUT)  # slot0 = C2
        else:
            b[0].enable_alu(AluOp.MULTIPLY, AluInp.PREV_ALU_OUT, AluInp.NEXT_ALU_OUT_A)
        b[1].enable_alu(AluOp.ADD, AluInp.PREV_ALU_OUT, AluInp.PREV_DELAY_0)
        b[1].alu_out_a_enable = ENABLE
        if boundary:
            b[2].enable_alu(AluOp.SUBTRACT, AluInp.PREV_DELAY_3, AluInp.PREV_DELAY_2)
        else:
            b[2].enable_alu(AluOp.MULTIPLY, AluInp.PREV_DELAY_2, AluInp.NEXT_ALU_OUT_A)
        b[3].enable_alu(AluOp.ADD, AluInp.PREV_ALU_OUT, AluInp.PREV_DELAY_4)
        b[3].alu_out_a_enable = ENABLE
        if boundary:
            b[4].enable_alu(AluOp.BYPASS, AluInp.PREV_DELAY_3)
        else:
            b[4].enable_alu(AluOp.MULTIPLY, AluInp.PREV_DELAY_1, AluInp.NEXT_ALU_OUT_A)
        if boundary:
            b[5].enable_alu(AluOp.BYPASS, AluInp.PREV_ALU_OUT)
        else:
            b[5].enable_alu(AluOp.ADD, AluInp.PREV_ALU_OUT, AluInp.NEXT_ALU_OUT_B)
        b[5].alu_out_a_enable = ENABLE
        b[6].enable_alu(AluOp.IS_LT, AluInp.PREV_ALU_OUT, AluInp.PREV_DELAY_5)
        b[6].alu_out_b_enable = ENABLE
        b[7].pass_through_alu()
        return b

    def mk(boundary):
        u = UopConfig()
        u.enable_input(InpSel.CONST_2 if boundary else InpSel.CONST_0, 0)
        u.enable_input(InpSel.SRC_0, LANE_XA + 1)
        u.enable_input(InpSel.CONST_0, LANE_DM + 1)
        u.enable_input(InpSel.CONST_1, LANE_DS + 1)
        u.enable_input(InpSel.ZERO, LANE_ZERO + 1)
        u.datapath_config = datapath(boundary)
        u.enable_output(OutSel.ALU_OUT, OutPath.WR0_LO)
        u.require_inp0 = 1
        if boundary:
            u.repeat_count = 2
            u.trigger = (Trigger.SRC_TENSOR_DONE, Trigger.SUB_DIM_DONE, Trigger.COUNT)
            u.next_uop = (0, 1, 2)
        else:
            u.trigger = (Trigger.SRC_TENSOR_DONE, Trigger.SUB_DIM_DONE, Trigger.NONE)
            u.next_uop = (0, 1, 0)
        return u

    return [mk(True), mk(True), mk(False)]


def _lif_ref_stream(xa, seg=SEG):
    """Numpy oracle of the fused op's stream semantics (CoreSim only)."""
    P, TOT = xa.shape
    x4 = xa.reshape(P, TOT // seg, seg // 2, 2)
    v = np.full(x4.shape[:2] + (2,), V_INIT, np.float32)
    h = np.full_like(v, np.float32(-1.0))
    F = np.zeros_like(v)
    o = np.zeros_like(v)
    out = np.zeros_like(x4)
    for t in range(seg // 2):
        vm = (DM * v).astype(np.float32)
        if t == 0:
            vm[...] = C2_VAL
        v = (vm + x4[:, :, t, :]).astype(np.float32)
        hm = (DS * h).astype(np.float32)
        h = (hm + v).astype(np.float32)
        Fm = (DM * F).astype(np.float32)
        F = (Fm + o).astype(np.float32)
        o = (F < h).astype(np.float32)
        out[:, :, t, :] = o
    return out.reshape(P, TOT)


def _build_pack_uops():
    """Weighted 8-element pack: y = sum(o[k]*w[k]) per subdim segment of 8,
    emitted once per segment (write_subdim_last).  1-cycle same-stage scan
    feedback; boundary uop (1 element) restarts the sum."""
    from concourse.dve_uop import (
        ENABLE,
        AluInp,
        AluOp,
        InpSel,
        OutPath,
        OutSel,
        Trigger,
        UopConfig,
        UopDpConfig,
    )

    def datapath(boundary):
        b = [UopDpConfig() for _ in range(8)]
        for st in range(8):
            b[st].pass_through_delay(0)
        b[0].enable_alu(AluOp.MULTIPLY, AluInp.PREV_ALU_OUT, AluInp.PREV_DELAY_0)
        if boundary:
            b[1].enable_alu(AluOp.BYPASS, AluInp.PREV_ALU_OUT)
        else:
            b[1].enable_alu(AluOp.ADD, AluInp.CURR_ALU_OUT, AluInp.PREV_ALU_OUT)
        for st in range(2, 8):
            b[st].pass_through_alu()
        return b

    def mk(boundary):
        u = UopConfig()
        u.enable_input(InpSel.SRC_0, 0)
        u.enable_input(InpSel.SRC_1, 1)
        u.datapath_config = datapath(boundary)
        u.enable_output(OutSel.ALU_OUT, OutPath.WR0_LO)
        u.out_last_subdim_enable = 1
        u.require_inp0 = 1
        u.require_inp1 = 1
        if boundary:
            u.repeat_count = 1
            u.trigger = (Trigger.SRC_TENSOR_DONE, Trigger.SUB_DIM_DONE, Trigger.COUNT)
            u.next_uop = (0, 1, 2)
        else:
            u.trigger = (Trigger.SRC_TENSOR_DONE, Trigger.SUB_DIM_DONE, Trigger.NONE)
            u.next_uop = (0, 1, 0)
        return u

    return [mk(True), mk(True), mk(False)]


def _register_ops():
    from concourse import dve_ops
    from concourse.dve_spec import C0, C1, C2, Spec, Src0, Src1
    from concourse.dve_uop import DveOpSpec

    def reg(name, uops, spec, rd1_en):
        for op in dve_ops.OPS:
            if op.name == name:
                return op
        row = dve_ops._CUSTOM_DVE_ROW_BASE + len(dve_ops.OPS)
        dve_ops._SUB_OPCODE_FOR_NAME[name] = row
        # uops_sha deliberately invalid: compile() must never fall through to
        # lower() -- the pre-populated cache below is the only source of uops.
        op = dve_ops.DveOp(
            name, spec, subdim=True,
            uops_sha={"v3": "PINNED-BY-CACHE", "v4": "PINNED-BY-CACHE"},
        )
        for ver in ("v3", "v4"):
            s = DveOpSpec(name=name, opcode=row, uops=uops, rd1_en=rd1_en)
            s.validate(ver)
            dve_ops._COMPILE_CACHE[(name, ver)] = s
        dve_ops.OPS.append(op)
        return op

    lif = reg(
        "LIF_FUSED_SCAN_ANT",
        _build_lif_uops(),
        Spec(
            body=Src0 * C0 + C1 + C2,  # placeholder: leaf set only
            reference=lambda in0, in1, s0, s1, imm2: _lif_ref_stream(
                in0.reshape(in0.shape[0], -1)
            ),
        ),
        rd1_en=False,
    )
    pack = reg(
        "PACK8_WSUM_ANT",
        _build_pack_uops(),
        Spec(
            body=Src0 * Src1,  # placeholder: leaf set only
            reference=lambda in0, in1, s0, s1, imm2: in0 * in1,
        ),
        rd1_en=True,
    )
    return lif, pack


# ------------------------------------------------------------- bass program
def _build_program(iters: int = 1, timing: bool = False, phases: str = "full"):
    import concourse.mybir as mybir
    from concourse import bacc, tile
    from contextlib import nullcontext

    do_in = phases in ("full", "in", "inA", "inAD")
    do_aff = phases in ("full", "inA", "inAD", "aff")
    do_dve = phases in ("full", "inAD", "dve")
    do_pack = phases in ("full", "pk")
    do_out = phases in ("full", "out")

    fp32 = mybir.dt.float32
    bf16 = mybir.dt.bfloat16
    op, pack_op = _register_ops()

    nc = bacc.Bacc("TRN2", target_bir_lowering=False, debug=False)
    if timing:
        # tiny external I/O + internal DRAM scratch: same on-device work,
        # no host<->device transfer noise in wall-clock measurements
        nc.dram_tensor("x", [128, T], fp32, kind="ExternalInput")
        o_ext = nc.dram_tensor("o", [128, 1], fp32, kind="ExternalOutput").ap()
        x_d = nc.dram_tensor("xs", [ROWS_PER_CORE, T], fp32).ap()
        o_d = nc.dram_tensor("os", [ROWS_PER_CORE // 2, SEG // 8], bf16).ap()
    else:
        x_d = nc.dram_tensor("x", [ROWS_PER_CORE, T], fp32,
                             kind="ExternalInput").ap()
        o_d = nc.dram_tensor("o", [ROWS_PER_CORE // 2, SEG // 8], bf16,
                             kind="ExternalOutput").ap()

    # partition-major row maps: long contiguous DMA runs per partition
    x_d3 = x_d.rearrange("(p gs) t -> p gs t", p=128)   # row = p*G + gs
    o_d3 = o_d.rearrange("(p s) n -> p s n", p=128)     # row = p*NPAIR + s

    with tile.TileContext(nc) as tc:
        with (
            tc.tile_pool(name="xb", bufs=3) as xb_pool,
            tc.tile_pool(name="xa", bufs=3) as xa_pool,
            tc.tile_pool(name="big", bufs=1) as big_pool,
        ):
            o_il = big_pool.tile([128, G * T], bf16)   # interleaved spikes
            p_il = big_pool.tile([128, G * T // 8], bf16)  # packed bytes
            w8 = big_pool.tile([128, 8], bf16)         # pack weights 128..1
            bias1 = big_pool.tile([128, 1], fp32)

            loop_cm = tc.For_i(0, iters, 1) if iters > 1 else nullcontext()
            with loop_cm:
                nc.vector.memset(bias1[:], float(ABIAS))
                for k in range(8):
                    nc.vector.memset(w8[:, k:k + 1], float(1 << (7 - k)))
                if do_pack and not do_dve:
                    nc.vector.memset(o_il[:, 0:1], 0.0)
                if do_out and not do_pack:
                    nc.vector.memset(p_il[:, 0:1], 0.0)
                for q in range(ND):                    # 4 chunks of 16 pairs
                    xa_t = xa_pool.tile([128, PB * SEG], fp32)
                    if do_dve and not do_aff:
                        nc.vector.memset(xa_t[:, 0:1], 0.0)
                    # interleaved view [p, s_local, j, t] for affine writes
                    xa_v = xa_t[:].rearrange(
                        "p (s t j) -> p s j t", t=T, j=2)
                    for b in range(PB * 2 // GB):      # 2 DMA batches / chunk
                        gb = q * (PB * 2 // GB) + b
                        xb = xb_pool.tile([128, GB * T], fp32)
                        if do_aff and not do_in:
                            nc.vector.memset(xb[:, 0:1], 0.0)
                        if do_in:
                            nc.sync.dma_start(
                                xb[:].rearrange("p (gs t) -> p gs t", t=T),
                                x_d3[:, gb * GB:(gb + 1) * GB, :])
                        xbv = xb[:].rearrange(
                            "p (s j t) -> p s j t", j=2, t=T)
                        dst = xa_v[:, b * (GB // 2):(b + 1) * (GB // 2)]
                        # affine xa = 0.6*x - 0.04, pair-interleave layout;
                        # alternate ScalarE / GPSIMD per batch
                        if not do_aff:
                            pass
                        elif gb % 2 == 0:
                            nc.scalar.activation(
                                dst, xbv,
                                mybir.ActivationFunctionType.Identity,
                                bias=bias1[:], scale=float(GAIN))
                        else:
                            nc.gpsimd.tensor_scalar(
                                dst, xbv, float(GAIN), float(ABIAS),
                                mybir.AluOpType.mult, mybir.AluOpType.add)
                    if do_dve:
                        nc.vector._custom_dve(
                            op,
                            out=o_il[:, q * PB * SEG:(q + 1) * PB * SEG],
                            in0=xa_t[:].rearrange("p (s n) -> p s n", n=SEG),
                            s0=float(DM), s1=float(DS), imm2=float(C2_VAL))
                    if do_pack:
                        nseg = PB * SEG // 8
                        nc.vector._custom_dve(
                            pack_op,
                            out=p_il[:, q * nseg:(q + 1) * nseg],
                            in0=o_il[:, q * PB * SEG:(q + 1) * PB * SEG]
                            .rearrange("p (s n) -> p s n", n=8),
                            in1=w8[:].rearrange("p (a n) -> p a n", a=1)
                            .broadcast_to([128, nseg, 8]))
                    if do_out:
                        nc.scalar.dma_start(
                            o_d3[:, q * PB:(q + 1) * PB, :],
                            p_il[:, q * PB * (SEG // 8):(q + 1) * PB * (SEG // 8)]
                            .rearrange("p (s n) -> p s n", n=SEG // 8))
                if timing:
                    nc.sync.dma_start(o_ext[:, :], bias1[:, :])

    nc.compile()
    return nc


def _run(x_flat: np.ndarray, iters: int = 1, trace: bool = False,
         phases: str = "full", timing: bool = False):
    from concourse.bass_utils import run_bass_kernel_spmd

    key = f"nc{iters}-{timing}-{phases}"
    if key not in _cached:
        _cached[key] = _build_program(iters, timing, phases)
    nc = _cached[key]
    if timing:
        in_maps = [{"x": np.zeros((128, T), np.float32)}
                   for _ in range(N_CORES)]
    else:
        shards = [
            np.ascontiguousarray(
                x_flat[c * ROWS_PER_CORE:(c + 1) * ROWS_PER_CORE])
            for c in range(N_CORES)
        ]
        in_maps = [{"x": s} for s in shards]
    res = run_bass_kernel_spmd(nc, in_maps, list(range(N_CORES)), trace=trace)
    if timing:
        return None, res
    outs = []
    for r in res.results:
        # [8192, 50] bf16 byte-values; row = p*NPAIR + s, 8 stream bits/byte
        od2 = np.asarray(r["o"], dtype=np.float32).astype(np.uint8)
        bits = np.unpackbits(od2, axis=-1)          # [8192, 400] stream order
        o4 = bits.reshape(128, NPAIR, T, 2)          # [p, s, t, j]
        outs.append(np.ascontiguousarray(
            o4.transpose(0, 1, 3, 2)).reshape(ROWS_PER_CORE, T)
            .astype(np.float32))
    return np.concatenate(outs, axis=0), res


def kernel(x, decay_m=None, decay_s=None):
    x = np.asarray(x, dtype=np.float32)
    out_flat, _ = _run(x.reshape(ROWS, T))
    return out_flat.reshape(B, N, T)
